# revision 68
# baseline (speedup 1.0000x reference)
"""Multi-head attention (B=8, N=1024, C=768, H=12) on 8 Trainium2 NeuronCores.

Sharding: data-parallel over batch — one batch element per core, no collectives.

Per-core dataflow (all layouts chosen so NO on-chip transposes are needed):
  - Host pre-transposes x and the weights into contraction-on-partition layouts.
  - Q^T,K^T computed in [o, n] layout (o on partitions) in 512-col PSUM chunks;
    V in [n, o] layout per head-PAIR ([128, 130] tiles: 2x(64 d-cols + ones col),
    ones written by memset) so the attn@V matmul also produces softmax row-sums.
  - S^T[m, n] = K^T.T @ Q^T per head (contraction over d=64 on partitions).
  - P^T = exp(0.125 * S^T) on ScalarE (no max-subtraction: logits ~ N(0,1)).
  - O'[d, n] (+rowsum row) = V'aug.T @ P^T, accumulated over m-tiles in PSUM.
  - heads 0..9: normalize via 1/rowsum broadcast (PSUM row -> DRAM -> stride-0
    DMA) and multiply. Heads 10/11 (the last pair) skip normalization: their
    proj contributions are scaled on the output side, where 1/rowsum is a
    per-partition scalar (rowsums gathered PSUM->SBUF as [128, 8] via a
    partition-scatter DMA, reciprocal on DVE, tensor_scalar multiply).
  - projection accumulated per n-tile in 512/256-col PSUM chunks spread through
    the last two attention pairs; bias folded into the first accumulation add.
All matmuls run as float32r (full-rate single-pass) with fp32 PSUM accumulation.
PE is the critical engine: dummy warm-up matmuls burn the p-state ramp during
the input DMAs, and V/QK/proj work is queued as fine-grained filler between
S/O matmuls so PE never waits on the exp (ACT) pipeline.
"""

import numpy as np

_STATE = {}

B, N, C = 8, 1024, 768
H, D = 12, 64
KT = 6           # contraction tiles of 128 over C
P = 128
NT = N // P      # 8 n-tiles
PAIRS = H // 2   # 6 head pairs
VC = 3           # V weight chunks (2 pairs each)
DW = 4 * (D + 1)  # 260: ones-augmented V width per 2-pair chunk


def _patch_tile_drain():
    """This walrus build rejects >1 sem wait on a CTRL (Drain) instruction.

    TileContext's exit puts one wait per outstanding semaphore on the final SP
    Drain; redistribute them across single-wait NOPs preceding the drain.
    """
    import bass_rust
    import concourse.tile as tile
    from concourse.vector_clock import ScopedClock

    if getattr(tile.TileContext, "_ant_drain_patched", False):
        return

    SyncInfo = bass_rust.SyncInfo

    def _drain_and_barrier(self, tick_clock, wait_clock):
        nc = self.nc
        probe = nc.sync.nop(nofuse=True)
        wait_clock.add_sem_waits(
            probe.ins, ScopedClock({None: tick_clock.global_clock})
        )
        si = probe.ins.sync_info
        waits = list(si.on_wait or []) if si is not None else []
        updates = list(si.on_update or []) if si is not None else []
        if len(waits) > 1:
            probe.ins.sync_info = SyncInfo(on_wait=waits[:1], on_update=updates)
            for w in waits[1:]:
                extra = nc.sync.nop(nofuse=True)
                extra.ins.sync_info = SyncInfo(on_wait=[w], on_update=[])
        nc.sync.drain()

        nc.all_engine_barrier()
        assert self.sems is not None
        popped = nc._tile_sem_poison_stack.pop()
        assert popped is self._sem_poison
        nc.clear_and_free_semaphores(list(self.sems.allocated().values()))
        nc.all_engine_barrier()

    tile.TileContext._drain_and_barrier = _drain_and_barrier
    tile.TileContext._ant_drain_patched = True


def _split_multi_waits(nc):
    """This walrus build allows at most ONE sem wait per instruction.

    Tile's wait assignment routinely puts several; hoist all but the last onto
    single-wait NOPs inserted immediately before the instruction on the same
    engine (engines execute block instructions in order, so semantics are
    unchanged).
    """
    from concourse import mybir

    for fn in nc.m.functions:
        for bb in fn.blocks:
            out, changed = [], False
            for inst in bb.instructions:
                si = inst.sync_info
                waits = list(si.on_wait) if (si is not None and si.on_wait) else []
                if len(waits) > 1:
                    changed = True
                    for w in waits[:-1]:
                        nop = mybir.InstNoOp(
                            name=f"I-ws{nc.next_id()}",
                            engine=inst.engine,
                            bass_nofuse=True,
                            sync_info=mybir.SyncInfo(on_wait=[w], on_update=[]),
                        )
                        nc.register_instruction(nop)
                        out.append(nop)
                    inst.sync_info = mybir.SyncInfo(
                        on_wait=[waits[-1]], on_update=list(si.on_update or [])
                    )
                out.append(inst)
            if changed:
                bb.instructions = out


def _build_nc(trace_sim=False):
    from contextlib import ExitStack

    import concourse.bass as bass
    import concourse.tile as tile
    from concourse import mybir

    _patch_tile_drain()

    f32 = mybir.dt.float32
    f32r = mybir.dt.float32r

    nc = bass.Bass("TRN2", target_bir_lowering=False, debug=False, num_devices=1)

    f8 = mybir.dt.float8e4
    xT = nc.dram_tensor("xT", [KT, P, 2, N], f8, kind="ExternalInput").ap()
    wqk = nc.dram_tensor("wqk", [PAIRS, P, 2 * 3 * 2 * 256], f8,
                         kind="ExternalInput").ap()
    wv = nc.dram_tensor("wv", [VC, P, 2 * 3 * 2 * DW], f8,
                        kind="ExternalInput").ap()
    pT = nc.dram_tensor("pT", [P, KT, C], f32r, kind="ExternalInput").ap()
    bias = nc.dram_tensor("bias", [P, C], f32, kind="ExternalInput").ap()
    pt5hi = nc.dram_tensor("pt5hi", [D, C], f32r, kind="ExternalInput").ap()
    ones = nc.dram_tensor("ones", [P, 4], f32r, kind="ExternalInput").ap()
    y = nc.dram_tensor("y", [N, C], f32, kind="ExternalOutput").ap()

    Exp = mybir.ActivationFunctionType.Exp
    AluOp = mybir.AluOpType
    SCALE = float(D) ** -0.5

    with tile.TileContext(nc, trace_sim=trace_sim) as tc, ExitStack() as ctx:
        kilo = ctx.enter_context(tc.tile_pool(name="kilo", bufs=1))     # xT
        qkp = ctx.enter_context(tc.tile_pool(name="qk", bufs=4))
        wqkp = ctx.enter_context(tc.tile_pool(name="wqk", bufs=2))
        wvp = ctx.enter_context(tc.tile_pool(name="wv", bufs=3))
        vp = ctx.enter_context(tc.tile_pool(name="v", bufs=18))
        ptp = ctx.enter_context(tc.tile_pool(name="pt", bufs=4))
        op_ = ctx.enter_context(tc.tile_pool(name="op", bufs=6))        # O'
        tbp = ctx.enter_context(tc.tile_pool(name="tb", bufs=2))
        rbp = ctx.enter_context(tc.tile_pool(name="rb", bufs=2))
        outp = ctx.enter_context(tc.tile_pool(name="out", bufs=8))
        onep = ctx.enter_context(tc.tile_pool(name="one", bufs=1))
        bigp = ctx.enter_context(tc.tile_pool(name="big", bufs=1))      # pT
        drp = ctx.enter_context(tc.tile_pool(name="dr", bufs=2, space="DRAM"))
        stgp = ctx.enter_context(tc.tile_pool(name="stg", bufs=2))
        tmpp = ctx.enter_context(tc.tile_pool(name="tmp", bufs=4))
        pso = ctx.enter_context(tc.tile_pool(name="pso", bufs=1, space="PSUM"))
        psA = ctx.enter_context(tc.tile_pool(name="psA", bufs=1, space="PSUM"))
        psB = ctx.enter_context(tc.tile_pool(name="psB", bufs=1, space="PSUM"))
        ps2 = ctx.enter_context(tc.tile_pool(name="ps2", bufs=2, space="PSUM"))

        # ---- input DMAs: x halves split over SP/ACT queues, V weights on
        # DVE; ordered so pair-0 QK work can start as early as possible ----
        x8 = kilo.tile([P, KT, 2, N], f8, tag="kilo", name="x8")
        wq_tiles = {}

        def prefetch_wq(t, eng):
            if t not in wq_tiles:
                wq_t = wqkp.tile([P, 2, 3, 2, 256], f8, tag="wqk", name=f"wq_{t}")
                eng.dma_start(wq_t[:], wqk[t])
                wq_tiles[t] = wq_t

        prefetch_wq(0, nc.sync)
        for k in range(KT):
            eng = nc.sync if k % 2 == 0 else nc.scalar
            eng.dma_start(x8[:, k, :, :], xT[k])

        # warm the ACT exp table set while input DMAs run (the first real exp
        # otherwise pays the ~2.7us ACT_TABLE_LOAD on the critical path)
        warm = onep.tile([1, 4], f32)
        nc.vector.memset(warm[:], 0.0)
        warm2 = onep.tile([1, 4], f32)
        nc.scalar.activation(warm2[:], warm[:], Exp)



        # V weights: chunk 0 (pairs 0-1) early on the Pool queue so V(0,*)
        # can start; chunks 1-2 later on the ACT queue
        wv_sb = [wvp.tile([P, 2, 3, 2, DW], f8, tag="wv", name=f"wvc_{c}")
                 for c in range(VC)]
        nc.gpsimd.dma_start(wv_sb[0][:], wv[0])

        # PE p-state pre-warm: dummy matmuls while the first inputs stream in
        dm_sb = onep.tile([P, 512], f32)
        nc.vector.memset(dm_sb[:], 0.0)
        for i in range(2):
            dslot = ps2.tile([P, 512], f32, tag="ps2", name=f"dwarm_{i}")
            nc.tensor.matmul(dslot[0:1, 0:128], dm_sb[:, 0:1], dm_sb[:, 0:128],
                             start=True, stop=True)

        bias_sb = onep.tile([P, C], f32)
        pt5hi_sb = onep.tile([D, C], f32r)
        rT11 = onep.tile([P, NT], f32r, name="rT11")

        # ---- QK chunks: one 512-col PSUM group per (pair, q/k, ns) ----
        qt_sb, kt_sb = {}, {}

        def qk_chunk(t, which, ns, act_copy=False):
            store = qt_sb if which == 0 else kt_sb
            if t not in store:
                store[t] = qkp.tile([P, N], f32r, tag="qk",
                                    name=f"{'q' if which == 0 else 'k'}_{t}")
            wq_t = wq_tiles[t]
            DR = mybir.MatmulPerfMode.DoubleRow
            sm = ps2.tile([P, 512], f32, tag="ps2", name=f"qm_{t}_{which}_{ns}")
            sl = ps2.tile([P, 512], f32, tag="ps2", name=f"ql_{t}_{which}_{ns}")
            ncol = slice(ns * 512, (ns + 1) * 512)
            wcol = slice(which * P, (which + 1) * P)
            for kp in range(3):
                nc.tensor.matmul(
                    sm[:, 0:512], wq_t[:, 0, kp, :, wcol],
                    x8[:, 2 * kp : 2 * kp + 2, 0, ncol],
                    start=(kp == 0), stop=(kp == 2), perf_mode=DR,
                )
            for kp in range(3):
                nc.tensor.matmul(
                    sl[:, 0:512], wq_t[:, 1, kp, :, wcol],
                    x8[:, 2 * kp : 2 * kp + 2, 0, ncol],
                    start=(kp == 0), stop=False, perf_mode=DR,
                    skip_group_check=True,
                )
                nc.tensor.matmul(
                    sl[:, 0:512], wq_t[:, 0, kp, :, wcol],
                    x8[:, 2 * kp : 2 * kp + 2, 1, ncol],
                    start=False, stop=(kp == 2), perf_mode=DR,
                    skip_group_check=True,
                )
            dest = store[t][:, ns * 512 : (ns + 1) * 512]
            if act_copy:
                # startup only: ACT is idle before the first exp, so the
                # hi-psum move comes off the serialized DVE epilogue chain
                nc.scalar.copy(dest, sm[:, 0:512])
            else:
                nc.vector.tensor_copy(dest, sm[:, 0:512])
            with nc.allow_low_precision(reason="f32r is f32 bits"):
                nc.vector.scalar_tensor_tensor(
                    dest, sl[:, 0:512], 1.0 / 16.0, dest,
                    op0=AluOp.mult, op1=AluOp.add,
                )

        # ---- V chunks: per (2-pair chunk, n-tile), [128, 260] ones-augmented ----
        v_sb = {}

        def v_chunk(c, j):
            DR = mybir.MatmulPerfMode.DoubleRow
            jcol = slice(j * P, (j + 1) * P)
            sm = ps2.tile([P, DW], f32, tag="ps2", name=f"vm_{c}_{j}")
            sl = ps2.tile([P, DW], f32, tag="ps2", name=f"vl_{c}_{j}")
            for kp in range(3):
                nc.tensor.matmul(
                    sm[:, 0:DW], x8[:, 2 * kp : 2 * kp + 2, 0, jcol],
                    wv_sb[c][:, 0, kp, :, :],
                    start=(kp == 0), stop=(kp == 2), perf_mode=DR,
                )
            for kp in range(3):
                nc.tensor.matmul(
                    sl[:, 0:DW], x8[:, 2 * kp : 2 * kp + 2, 1, jcol],
                    wv_sb[c][:, 0, kp, :, :],
                    start=(kp == 0), stop=False, perf_mode=DR,
                    skip_group_check=True,
                )
                nc.tensor.matmul(
                    sl[:, 0:DW], x8[:, 2 * kp : 2 * kp + 2, 0, jcol],
                    wv_sb[c][:, 1, kp, :, :],
                    start=False, stop=(kp == 2), perf_mode=DR,
                    skip_group_check=True,
                )
            vt = vp.tile([P, DW], f32r, tag="v", name=f"v_{c}_{j}")
            nc.vector.tensor_copy(vt[:], sm[:, 0:DW])
            with nc.allow_low_precision(reason="f32r is f32 bits"):
                nc.vector.scalar_tensor_tensor(
                    vt[:], sl[:, 0:DW], 1.0 / 16.0, vt[:],
                    op0=AluOp.mult, op1=AluOp.add,
                )
            ones_ap = vt.rearrange("p (h w) -> p h w", w=D + 1)[:, :, D]
            nc.sync.dma_start(ones_ap, ones[:])
            v_sb[(c, j)] = vt

        # pair-0 QK immediately (chasing the input DMA arrivals)
        qk_chunk(0, 0, 0, act_copy=True)
        qk_chunk(0, 1, 0, act_copy=True)
        qk_chunk(0, 0, 1, act_copy=True)
        qk_chunk(0, 1, 1)

        # ---- filler queue: chunks of PE work interleaved into attention ----
        fillers = []

        def pop_filler(budget):
            for _ in range(budget):
                if fillers:
                    fillers.pop(0)()

        # ---- projection accumulation helpers ----
        pt_w = [None]
        acc_sb = {}

        def partial_chunk(nt, half, ks):
            # cols 0:512 (half 0) / 512:768 (half 1) of proj, two k-tiles
            c0, w = (0, 512) if half == 0 else (512, 256)
            slot = ps2.tile([P, 512], f32, tag="ps2", name=f"pp_{nt}_{half}_{ks[0]}")
            for k in ks:
                nc.tensor.matmul(
                    slot[:, 0:w],
                    o_sb[k][:, nt * P : (nt + 1) * P],
                    pt_w[0][:, k, c0 : c0 + w],
                    start=(k == ks[0]),
                    stop=(k == ks[-1]),
                    skip_group_check=True,
                )
            if nt not in acc_sb:
                acc_sb[nt] = outp.tile([P, C], f32, tag="out", name=f"acc_{nt}")
            eng = nc.vector
            if ks[0] == 0:
                eng.tensor_add(acc_sb[nt][:, c0 : c0 + w], slot[:, 0:w],
                               bias_sb[:, c0 : c0 + w])
            else:
                eng.tensor_add(acc_sb[nt][:, c0 : c0 + w],
                               acc_sb[nt][:, c0 : c0 + w], slot[:, 0:w])

        def finish_chunk(nt, half):
            # k-tile 4 plus head-10's rows of k-tile 5 of the projection;
            # runs post-exp: ACT moves PSUM->SBUF, Pool accumulates (keeps
            # DVE free for the fused head-11 multiply-accumulate)
            c0, w = (0, 512) if half == 0 else (512, 256)
            slot = ps2.tile([P, 512], f32, tag="ps2", name=f"pf_{nt}_{half}")
            nc.tensor.matmul(
                slot[:, 0:w],
                o_sb[4][:, nt * P : (nt + 1) * P],
                pt_w[0][:, 4, c0 : c0 + w],
                start=True, stop=False, skip_group_check=True,
            )
            nc.tensor.matmul(
                slot[:, 0:w],
                o_sb[5][0:D, nt * P : (nt + 1) * P],
                pt_w[0][0:D, 5, c0 : c0 + w],
                start=False, stop=True, skip_group_check=True,
            )
            tmp = tmpp.tile([P, 512], f32, tag="tmp", name=f"tm_{nt}_{half}")
            nc.scalar.copy(tmp[:, 0:w], slot[:, 0:w])
            nc.gpsimd.tensor_add(acc_sb[nt][:, c0 : c0 + w],
                                 acc_sb[nt][:, c0 : c0 + w], tmp[:, 0:w])

        def gather_rowsums(stg, rt):
            # staged rowsum row -> per-n-tile [128, 1] partition-scatter
            # DMAs + reciprocals ([n] lands on partitions: a per-partition
            # scalar for the output-side scale)
            for nt in range(NT):
                piece = stg[D : D + 1, nt * P : (nt + 1) * P]
                eng = nc.sync if nt % 2 == 0 else nc.scalar
                eng.dma_start(out=rt[:, nt : nt + 1], in_=piece)
                with nc.allow_low_precision(reason="f32r is f32 bits"):
                    nc.vector.reciprocal(rt[:, nt : nt + 1], rt[:, nt : nt + 1])

        # ---- attention ----
        o_sb = []          # per pair (0..4): [P, N] (two heads stacked)

        def emit_S(t, head, j):
            hb = head * D
            pool, tg = (psA, "psA") if j % 2 == 0 else (psB, "psB")
            s_slot = pool.tile([P, N], f32, tag=tg, name=f"s_{2*t+head}_{j}")
            for ns in range(2):
                nc.tensor.matmul(
                    s_slot[:, ns * 512 : (ns + 1) * 512],
                    kt_sb[t][hb : hb + D, j * P : (j + 1) * P],
                    qt_sb[t][hb : hb + D, ns * 512 : (ns + 1) * 512],
                    start=True, stop=True,
                )
            return s_slot

        def stage_o(o_slot, h):
            # copy O' (+rowsum row) out of PSUM immediately so the single
            # o-slot frees before the bounce-broadcast latency
            stg = stgp.tile([D + 1, N], f32r, tag="stg", name=f"stg_{h}")
            nc.vector.tensor_copy(stg[:], o_slot[0 : D + 1, :])
            return stg

        def normalize(t, head, stg):
            h = 2 * t + head
            rb = rbp.tile([D, N], f32r, tag="rb", name=f"rb_{h}")
            with nc.allow_low_precision(reason="f32r is f32 bits"):
                nc.vector.reciprocal(stg[D : D + 1, :], stg[D : D + 1, :])
            scratch = drp.tile([1, N], f32r, tag="dr", name=f"sc_{h}")
            nc.sync.dma_start(scratch[0:1, :], stg[D : D + 1, :])
            bcast_src = bass.AP(
                tensor=scratch.tensor,
                offset=scratch.offset,
                ap=[[0, D]] + [list(dd) for dd in scratch[0:1, :].ap[1:]],
            )
            nc.gpsimd.dma_start(out=rb[:], in_=bcast_src)
            if head == 0:
                nc.gpsimd.tensor_mul(o_sb[t][0:D, :], stg[0:D, :], rb[:])
            else:
                tb = tbp.tile([D, N], f32r, tag="tb", name=f"tb_{h}")
                nc.gpsimd.tensor_mul(tb[:], stg[0:D, :], rb[:])
                nc.sync.dma_start(o_sb[t][D:P, :], tb[:])

        pre_s = [[]]
        for t in range(PAIRS):
            ot = op_.tile([P, N], f32r, tag="op", name=f"ot_{t}")
            o_sb.append(ot)
            # filler pushes for this pair
            if t < PAIRS - 1:
                prefetch_wq(t + 1, nc.sync)
                for which in range(2):
                    for ns in range(2):
                        fillers.append(lambda t=t, w=which, n=ns: qk_chunk(t + 1, w, n))
            if t in (0, 2):
                c = t // 2 + 1
                nc.sync.dma_start(wv_sb[c][:], wv[c])
            if t < 4:
                c = t // 2 + 1
                js = range(0, 4) if t % 2 == 0 else range(4, NT)
                for j in js:
                    fillers.append(lambda c=c, j=j: v_chunk(c, j))
            if t == 5:
                for nt in range(4, NT):
                    for half in range(2):
                        fillers.append(lambda nt=nt, h=half: partial_chunk(nt, h, (2, 3)))
            if t == 1:
                pt_w_t = bigp.tile([P, KT, C], f32r, tag="big", name="pt_w")
                nc.gpsimd.dma_start(pt_w_t[:], pT[:])
                pt_w[0] = pt_w_t
                nc.sync.dma_start(bias_sb[:], bias[:])
                nc.sync.dma_start(pt5hi_sb[:], pt5hi[:])
            if t == 3:
                for nt in range(NT):
                    for half in range(2):
                        fillers.append(lambda nt=nt, h=half: partial_chunk(nt, h, (0, 1)))
            if t == 4:
                for nt in range(4):
                    for half in range(2):
                        fillers.append(lambda nt=nt, h=half: partial_chunk(nt, h, (2, 3)))

            for head in range(2):
                h = 2 * t + head
                o_slot = pso.tile([P, N], f32, tag="pso", name=f"o_{h}")
                vb = h - 4 * (t // 2)
                nxt = (t + (head + 1) // 2, 1 - head) if h < 11 else None

                def emit_exp(j, s_slot):
                    # emitted immediately after its S matmuls so the exp's
                    # PE-clock wait covers ONLY the S (Tile uses one monotone
                    # counter per engine; anything emitted in between would
                    # falsely gate the exp)
                    pt_t = ptp.tile([P, N], f32r, tag="pt", name=f"p_{h}_{j}")
                    nc.scalar.activation(pt_t[:], s_slot[:], Exp, scale=SCALE)
                    return pt_t

                if pre_s[0]:
                    s0, s1 = pre_s[0]
                else:
                    s0, s1 = emit_S(t, head, 0), emit_S(t, head, 1)
                pre_s[0] = []
                pts = [emit_exp(0, s0), emit_exp(1, s1)]
                if t == 0 and head == 0:
                    v_chunk(0, 0)
                    v_chunk(0, 1)
                for j in range(NT):
                    if j + 2 < NT:
                        pts.append(emit_exp(j + 2, emit_S(t, head, j + 2)))
                    elif nxt is not None:
                        # pre-emit the next head's S_0/S_1 here: their
                        # slot-WAR deps are this head's j6/j7 exps, so the
                        # next head's exp stream starts without a boundary
                        # stall
                        pre_s[0].append(emit_S(nxt[0], nxt[1], j - 6))
                    for ns in range(2):
                        nc.tensor.matmul(
                            o_slot[0 : D + 1, ns * 512 : (ns + 1) * 512],
                            v_sb[(t // 2, j)][:, vb * (D + 1) : (vb + 1) * (D + 1)],
                            pts[j][:, ns * 512 : (ns + 1) * 512],
                            start=(j == 0),
                            stop=(j == NT - 1),
                            skip_group_check=True,
                        )
                    if t == 0 and head == 0:
                        if j + 2 < NT:
                            v_chunk(0, j + 2)
                    elif h == 11:
                        pop_filler(1)
                    else:
                        pop_filler(2 if (head == 1 or t == 5) else 1)
                stg = stage_o(o_slot, h)
                if h < 11:
                    normalize(t, head, stg)
                else:
                    stg11 = stg
            if t < 4:
                pop_filler(len(fillers))

        # ---- tail: head 11's proj contribution, interleaved with the
        # leftover finish/partial chunks across all free PSUM slots ----
        gather_rowsums(stg11, rT11)
        from concourse.mybir import AluOpType
        tail_pools = [(pso, "pso"), (psA, "psA"), (psB, "psB")]
        for nt in range(NT):
            pop_filler(1)
            finish_chunk(nt, 0)
            finish_chunk(nt, 1)
            pool, tg = tail_pools[nt % 3]
            slot = pool.tile([P, N], f32, tag=tg, name=f"pb_{nt}")
            for c0, w in ((0, 512), (512, 256)):
                nc.tensor.matmul(
                    slot[:, c0 : c0 + w],
                    stg11[0:D, nt * P : (nt + 1) * P],
                    pt5hi_sb[:, c0 : c0 + w],
                    start=True, stop=True, skip_group_check=True,
                )
            nc.vector.scalar_tensor_tensor(
                acc_sb[nt][:], slot[:, 0:C],
                rT11[:, nt : nt + 1].bitcast(f32), acc_sb[nt][:],
                op0=AluOpType.mult, op1=AluOpType.add,
            )
            nc.sync.dma_start(y[nt * P : (nt + 1) * P, :], acc_sb[nt][:])
        pop_filler(len(fillers))

    _split_multi_waits(nc)
    return nc


def _f8split(a):
    import ml_dtypes

    E4 = ml_dtypes.float8_e4m3fn
    hi = a.astype(E4)
    lo = ((a - hi.astype(np.float32)) * 16.0).astype(E4)
    return hi, lo


def _prep_shared(qkv_w, proj_w, proj_b):
    f = np.float32
    wq = qkv_w[0:C].astype(f)          # [o, c]
    wk = qkv_w[C : 2 * C].astype(f)
    wv_ = qkv_w[2 * C : 3 * C].astype(f)
    wqT, wkT, wvT = wq.T.copy(), wk.T.copy(), wv_.T.copy()  # [c, o]

    import ml_dtypes

    E4 = ml_dtypes.float8_e4m3fn
    wqk = np.zeros((PAIRS, P, 2, 3, 2, 256), E4)
    for t in range(PAIRS):
        for kp in range(3):
            for pe in range(2):  # k-tile within the DoubleRow pair
                k = 2 * kp + pe
                qh, ql = _f8split(wqT[k * P : (k + 1) * P, t * P : (t + 1) * P])
                kh, kl = _f8split(wkT[k * P : (k + 1) * P, t * P : (t + 1) * P])
                wqk[t, :, 0, kp, pe, 0:P] = qh
                wqk[t, :, 1, kp, pe, 0:P] = ql
                wqk[t, :, 0, kp, pe, P:256] = kh
                wqk[t, :, 1, kp, pe, P:256] = kl
    wqk = wqk.reshape(PAIRS, P, 2 * 3 * 2 * 256)

    wvh = np.zeros((VC, P, 2, 3, 2, 4, D + 1), E4)
    for c in range(VC):
        for kp in range(3):
            for pe in range(2):
                k = 2 * kp + pe
                for hh in range(4):
                    h = 4 * c + hh
                    vh, vl = _f8split(
                        wvT[k * P : (k + 1) * P, h * D : (h + 1) * D]
                    )
                    wvh[c, :, 0, kp, pe, hh, 0:D] = vh
                    wvh[c, :, 1, kp, pe, hh, 0:D] = vl
    wvh = wvh.reshape(VC, P, 2 * 3 * 2 * DW)

    pTh = proj_w.T.astype(f).reshape(KT, P, C).transpose(1, 0, 2).copy()
    pt5hi = np.ascontiguousarray(proj_w.T.astype(f)[C - D : C, :])
    bias_h = np.ascontiguousarray(np.broadcast_to(proj_b.astype(f), (P, C)))
    return wqk, wvh, pTh, bias_h, pt5hi


def kernel(x, qkv_w, proj_w, proj_b):
    from concourse.bass_utils import run_bass_kernel_spmd

    x = np.asarray(x, np.float32)
    wqk, wvh, pTh, bias_h, pt5hi = _prep_shared(
        np.asarray(qkv_w), np.asarray(proj_w), np.asarray(proj_b)
    )

    if "nc" not in _STATE:
        _STATE["nc"] = _build_nc()
    nc = _STATE["nc"]

    in_maps = []
    for b in range(B):
        xf = np.ascontiguousarray(x[b].T).reshape(KT, P, N)
        xh, xl = _f8split(xf)
        xTb = np.stack([xh, xl], axis=2)  # [KT, P, 2, N] fp8
        in_maps.append(
            {"xT": xTb, "wqk": wqk, "wv": wvh, "pT": pTh, "bias": bias_h,
             "pt5hi": pt5hi, "ones": np.ones((P, 4), np.float32)}
        )

    res = run_bass_kernel_spmd(nc, in_maps, core_ids=list(range(B)))
    return np.stack([res.results[b]["y"] for b in range(B)], axis=0)
